# revision 27
# baseline (speedup 1.0000x reference)
"""Trainium2 Bass kernel for the DNPU local-receptive-field surrogate model.

Model (see reference): x [B,1,64,64] -> 2x2/stride-2 unfold -> per-node
7-electrode assembly -> shared MLP 7->90->(90x4)->1 -> out [B,32,32].

Measured hardware facts driving the design (this axon-tunneled TRN2):
  - A bf16 matmul streams 1 moving column per 1.2GHz cycle (426.7ns per
    512-column tile), regardless of dtype, M, K, and runtime (no 2.4GHz
    p-state ever engages; verified with 2000 dependency-free matmuls).
  - Matmuls whose outputs sit in DIFFERENT 32-partition PSUM column
    blocks (matmul tile_position=(0, 32j)) execute CONCURRENTLY; tiles
    sharing a column block serialize. So PE throughput = columns per
    block-saturated schedule, NOT per instruction.
  - ACT/DVE (the only PSUM readers) are free-dim-serial: a [128,512]
    drain costs the same as [1,512] (~620ns), so drains want ALL used
    partitions per instruction.
  - fp8 DoubleRow is ISA-rejected for this geometry and ~12% rel err
    anyway (budget 2e-2); bf16 keeps rel err at 6.4e-3.

Strategy:
  - Data-parallel over batch: 64 batches/core x 8 cores; weights and
    per-node controls replicated; host pre-packs x into pixel-major
    [7, tokens] (4 patch pixels + 3 tiled control rows) so unfold +
    electrode scatter + layer-0 is one K=7 matmul per 512-token tile.
  - SOFTWARE-PIPELINED BLOCK-ROTATED SWEEPS: each relu layer s places
    its 90 output rows on 3 of the 4 PSUM column blocks, rotated per
    layer (ROT[s]); the next layer's stationary is host-permuted to
    [128, 90] with zero rows where the producer layout has garbage, so
    K=128 matmuls consume the placed layout directly and a single
    [128, 512] drain (placed bias + relu; junk rows drain harmlessly,
    zero stationary rows kill them downstream) retires each tile.
    A per-chunk round-robin emitter interleaves all 5 layer sweeps
    (+ the M=1 out matmuls, 4 per block-packed PSUM slot) with a lag,
    so on average all 4 column blocks stay loaded: per-block demand is
    15 layer-parts + out spread over 4 blocks -> ~342ns/tile-layer vs
    427 serialized, on top of the out layer costing ~1/4 of a sweep.
  - Out groups: 4 M=1 matmuls -> partitions {0,32,64,96} of one PSUM
    slot -> one [97,512] drain -> one 4-partition-line DMA (the old
    [1, chunk_tok] single-line output DMA was a ~25us serial transfer
    that dominated the tail).
  - Chunk-0 x arrives in staged pieces so layer 0 starts once ~28KB
    land; later chunks prefetch after layer 0 stops reading xt.
"""

import ml_dtypes
import numpy as np

import concourse.bass as bass
import concourse.mybir as mybir
import concourse.tile as _tile
from concourse.bass_utils import run_bass_kernel_spmd

# ---------------------------------------------------------------------------
# Workaround: this neuronxcc walrus build rejects instructions carrying more
# than one sem wait ("Too many sync wait commands"). Spill excess waits onto
# NOPs inserted just before the instruction on the same engine.
_MAX_SYNC_WAITS = 1
_nop_counter = [0]


def _split_excess_sync_waits(nc, maxw=_MAX_SYNC_WAITS):
    for f in nc.m.functions:
        for bb in f.blocks:
            insts = list(bb.instructions)
            if not any(
                ins.sync_info is not None and len(ins.sync_info.on_wait or []) > maxw
                for ins in insts
            ):
                continue
            new = []
            for ins in insts:
                si = ins.sync_info
                waits = list(si.on_wait or []) if si is not None else []
                if len(waits) > maxw:
                    excess, keep = waits[: len(waits) - maxw], waits[-maxw:]
                    for i in range(0, len(excess), maxw):
                        _nop_counter[0] += 1
                        nop = mybir.InstNoOp(name=f"waitsplit_{_nop_counter[0]}")
                        nop.engine = ins.engine
                        nop.sync_info = mybir.SyncInfo(
                            on_wait=excess[i : i + maxw], on_update=[]
                        )
                        new.append(nop)
                    si.on_wait = keep
                new.append(ins)
            bb.instructions = new

# ---------------------------------------------------------------------------
B = 512
H = W = 64
K = 2
N_NODES = (H // K) * (W // K)  # 1024
HID = 90
N_HIDDEN = 4
N_STAGES = 1 + N_HIDDEN  # relu layers (l0..l4)
N_CORES = 8
B_CORE = B // N_CORES  # 64 batches per core

CHUNK_B = 16
N_TILE = 512
OGRP = 4  # out tiles per PSUM slot

# per-relu-layer column blocks for parts P0 (hid 0:32), P1 (32:64),
# P2 (64:90); rotated so the 15 parts spread over 4 blocks
ROT = [[0, 1, 2], [3, 0, 1], [2, 3, 0], [1, 2, 3], [0, 1, 2]]
PART_W = [32, 32, HID - 64]  # part widths
PART_H0 = [0, 32, 64]  # hid range starts
LAG_T = 5  # consumer stage trails producer by this many tiles

F32 = mybir.dt.float32
BF16 = mybir.dt.bfloat16

_COST = {
    "act_out": 612.0,
    "dve_out": 700.0,  # measured 658 + DVE sem/NOP overhead share (792 and 760 both measured worse)
    "act_pair": 1038.0,
    "dve_pair": 1230.0,
}


def _hid_of_row(s, v):
    """Relu layer s, tile-variant v output layout: psum row r -> hid or
    None. Variant v rotates all three blocks by v so per-block PE load
    averages out to 3.75 layer-parts + 1/4 out stream per tile slot."""
    m = [None] * 128
    for j, blk in enumerate(ROT[s]):
        for k in range(PART_W[j]):
            m[32 * ((blk + v) % 4) + k] = PART_H0[j] + k
    return m


def _build_program(b_core: int, chunk_b: int):
    nc = bass.Bass()

    n_chunks = b_core // chunk_b
    chunk_tok = chunk_b * N_NODES
    tiles = chunk_tok // N_TILE  # 32
    ogroups = tiles // OGRP  # 8

    xs_d = nc.dram_tensor("xs", [7, b_core * N_NODES], BF16, kind="ExternalInput")
    wz_d = nc.dram_tensor("wz", [7, HID], BF16, kind="ExternalInput")
    # permuted hidden stationaries [128, layer, variant, 90]
    wh_d = nc.dram_tensor(
        "wh", [128, N_HIDDEN, 4, HID], BF16, kind="ExternalInput"
    )
    # permuted out stationaries per variant [128, 4]
    wo_d = nc.dram_tensor("wo", [128, 4], BF16, kind="ExternalInput")
    # placed biases [128, layer, variant]
    bia_d = nc.dram_tensor("bia", [128, N_STAGES, 4], F32, kind="ExternalInput")
    boc_d = nc.dram_tensor("boc", [97, 1], F32, kind="ExternalInput")
    z_d = nc.dram_tensor("z", [1, 640], BF16, kind="ExternalInput")
    out_d = nc.dram_tensor("out", [b_core, N_NODES], F32, kind="ExternalOutput")

    Relu = mybir.ActivationFunctionType.Relu
    Identity = mybir.ActivationFunctionType.Identity
    ALU_ADD = mybir.AluOpType.add
    ALU_MAX = mybir.AluOpType.max

    eng_t = [0.0, 0.0]

    with _tile.TileContext(nc) as tc:
        with (
            tc.tile_pool(name="const", bufs=1) as const,
            tc.tile_pool(name="xin", bufs=1) as xin,
            tc.tile_pool(name="outp", bufs=2 * ogroups) as outp,
            tc.tile_pool(name="hbuf", bufs=60) as hbuf,
            tc.tile_pool(name="ps", bufs=8, space="PSUM") as ps,
        ):
            # z first: the PSUM scrub matmuls below depend only on zt,
            # and must finish before the first layer-0 drain; issuing z
            # first lets the ~3.4us serialized scrub hide under the
            # remaining DMA issue/transfer time instead of gating the
            # whole pipeline start.
            zt = const.tile([1, 640], BF16)
            nc.sync.dma_start(zt[:], z_d[:])
            wz = const.tile([7, HID], BF16)
            nc.sync.dma_start(wz[:], wz_d[:])
            # double-buffered chunk input; chunk 0 staged in pieces so
            # layer 0 starts as soon as ~28KB land
            xts = []
            for cb in range(2):
                xts.append(xin.tile([7, chunk_tok], BF16, name=f"xt{cb}"))
            bounds = [0, 4, 8, 14, 22, 32]
            for a, b in zip(bounds, bounds[1:]):
                nc.sync.dma_start(
                    xts[0][:, a * N_TILE : b * N_TILE],
                    xs_d[:, a * N_TILE : b * N_TILE],
                )
            nc.sync.dma_start(xts[1][:], xs_d[:, chunk_tok : 2 * chunk_tok])
            bia = const.tile([128, N_STAGES, 4], F32)
            nc.sync.dma_start(bia[:], bia_d[:])
            wh = const.tile([128, N_HIDDEN, 4, HID], BF16)
            nc.sync.dma_start(wh[:], wh_d[:])
            wo = const.tile([128, 4], BF16)
            nc.sync.dma_start(wo[:], wo_d[:])
            boc = const.tile([97, 1], F32)
            nc.sync.dma_start(boc[:], boc_d[:])

            # scrub all 8 PSUM ring slots to 0 before any [128,512] drain
            # can observe boot-time garbage (0 x NaN = NaN would otherwise
            # poison the zero-stationary-row trick); overlaps the x DMA.
            for _ in range(8):
                pz = ps.tile([128, N_TILE], F32, tag="ps", name="pz")
                nc.tensor.matmul(
                    pz[:], zt[0:1, 0:128], zt[0:1, 128 : 128 + N_TILE]
                )

            def drain(dst, src, bias_ap, relu, kind="out", eng=None):
                ca, cd = _COST[f"act_{kind}"], _COST[f"dve_{kind}"]
                if eng is None:
                    use_act = eng_t[0] + ca <= eng_t[1] + cd
                else:
                    use_act = eng == 0
                if use_act:
                    eng_t[0] += ca
                    nc.scalar.activation(
                        dst, src, Relu if relu else Identity, bias=bias_ap
                    )
                elif relu:
                    eng_t[1] += cd
                    nc.vector.tensor_scalar(
                        out=dst, in0=src, scalar1=bias_ap, scalar2=0.0,
                        op0=ALU_ADD, op1=ALU_MAX,
                    )
                else:
                    eng_t[1] += cd
                    nc.vector.tensor_scalar(
                        out=dst, in0=src, scalar1=bias_ap, scalar2=None,
                        op0=ALU_ADD,
                    )

            # ---- single global software-pipelined emission over all
            # chunks: stage s's tile pointer runs over 0..n_tiles_total,
            # trailing its producer by LAG_T tiles; l0 switches xt
            # buffers at chunk boundaries and triggers the next
            # prefetch as soon as it finishes reading a chunk.
            n_tiles_total = n_chunks * tiles
            n_ogroups_total = n_tiles_total // OGRP
            hcur = [[None] * n_tiles_total for _ in range(N_STAGES)]

            def emit_stage_tile(s, t):
                """3 part-matmuls + 1 drain for relu layer s, tile t."""
                pt = ps.tile([128, N_TILE], F32, tag="ps", name="pt_s")[:]
                # v=0 pins the per-layer ROT layout: per-tile rotation
                # measured net-neutral-to-negative (drains, not PE block
                # imbalance, are the binding resource at this point)
                v = 0
                for j, blk0 in enumerate(ROT[s]):
                    w = PART_W[j]
                    blk = (blk0 + v) % 4
                    if s == 0:
                        lhs = wz[:, PART_H0[j] : PART_H0[j] + w]
                        tl = t % tiles
                        rhs = xts[(t // tiles) % 2][
                            :, tl * N_TILE : (tl + 1) * N_TILE
                        ]
                    else:
                        lhs = wh[:, s - 1, v, PART_H0[j] : PART_H0[j] + w]
                        rhs = hcur[s - 1][t][:]
                    nc.tensor.matmul(
                        pt[32 * blk : 32 * blk + w, :],
                        lhs,
                        rhs,
                        tile_position=(0, 32 * blk),
                    )
                h = hbuf.tile([128, N_TILE], BF16, tag="h", name="h_s")
                drain(h[:], pt, bia[:, s, v : v + 1], relu=True)
                hcur[s][t] = h

            def emit_out_group(g):
                """4 M=1 out matmuls -> one psum slot -> drain + DMA."""
                pt = ps.tile([128, N_TILE], F32, tag="ps", name="pt_o")
                for j in range(OGRP):
                    t = g * OGRP + j
                    nc.tensor.matmul(
                        pt[32 * j : 32 * j + 1, :],
                        wo[:, 0:1],
                        hcur[N_STAGES - 1][t][:],
                        tile_position=(0, 32 * j),
                    )
                og = outp.tile([97, N_TILE], F32, tag="o", name="og")
                drain(og[:], pt[0:97, :], boc[:], relu=False)
                dst = (
                    out_d[2 * g : 2 * g + 2]
                    .rearrange("b (r c) -> (b r) c", r=2, c=N_TILE)
                )
                nc.sync.dma_start(dst, og[0:97:32, :])

            p = [0] * N_STAGES
            po = 0
            next_prefetch = 2  # chunks 0,1 already issued
            while p[-1] < n_tiles_total or po < n_ogroups_total:
                progress = False
                for s in range(N_STAGES):
                    limit = n_tiles_total if s == 0 else p[s - 1] - LAG_T
                    if p[s] < n_tiles_total and p[s] < limit:
                        emit_stage_tile(s, p[s])
                        p[s] += 1
                        progress = True
                        if (
                            s == 0
                            and next_prefetch < n_chunks
                            and p[0] == (next_prefetch - 1) * tiles
                        ):
                            # l0 done reading chunk next_prefetch-2's
                            # buffer; refill it with chunk next_prefetch
                            nt0 = next_prefetch * chunk_tok
                            nc.sync.dma_start(
                                xts[next_prefetch % 2][:],
                                xs_d[:, nt0 : nt0 + chunk_tok],
                            )
                            next_prefetch += 1
                if po < n_ogroups_total and (po + 1) * OGRP <= p[-1] - LAG_T:
                    emit_out_group(po)
                    po += 1
                    progress = True
                if not progress:
                    for s in range(N_STAGES):
                        if p[s] < n_tiles_total and (
                            s == 0
                            or (
                                p[s] < p[s - 1]
                                and hcur[s - 1][p[s]] is not None
                            )
                        ):
                            emit_stage_tile(s, p[s])
                            p[s] += 1
                            break
                    else:
                        if po < n_ogroups_total:
                            emit_out_group(po)
                            po += 1

    _split_excess_sync_waits(nc)
    return nc


def _prep_weights(controls, W_in, b_in, W_h, b_h, W_out, b_out, data_idx, ctrl_idx):
    di = np.asarray(data_idx)[0].tolist()
    ci = np.asarray(ctrl_idx)[0].tolist()
    W_in = np.asarray(W_in, dtype=np.float32)
    Wd = W_in[di, :].copy()
    cset = set(ci)
    for j in range(4):
        if di[j] in cset or di[j] in di[j + 1 :]:
            Wd[j] = 0.0
    Wc = W_in[ci, :].copy()
    for k in range(3):
        if ci[k] in ci[k + 1 :]:
            Wc[k] = 0.0

    bf = ml_dtypes.bfloat16
    W_h = np.asarray(W_h, np.float32)  # [4, 90, 90]

    # permuted hidden stationaries: layer s (1..4), tile-variant v
    # consumes h_{s-1} laid out per (s-1, v); stationary row r multiplies
    # h row r (zero rows kill drained garbage)
    whp = np.zeros((128, N_HIDDEN, 4, HID), np.float32)
    for s in range(1, N_STAGES):
        for v in range(4):
            rowmap = _hid_of_row(s - 1, v)
            for r in range(128):
                hid = rowmap[r]
                if hid is not None:
                    whp[r, s - 1, v, :] = W_h[s - 1, hid, :]

    W_out = np.asarray(W_out, np.float32).ravel()
    wop = np.zeros((128, 4), np.float32)
    for v in range(4):
        rowmap = _hid_of_row(N_STAGES - 1, v)
        for r in range(128):
            if rowmap[r] is not None:
                wop[r, v] = W_out[rowmap[r]]

    # placed biases [128, 5, 4]
    b_in = np.asarray(b_in, np.float32)
    b_h = np.asarray(b_h, np.float32)  # [4, 90]
    bia = np.zeros((128, N_STAGES, 4), np.float32)
    for s in range(N_STAGES):
        for v in range(4):
            rowmap = _hid_of_row(s, v)
            bs = b_in if s == 0 else b_h[s - 1]
            for r in range(128):
                if rowmap[r] is not None:
                    bia[r, s, v] = bs[rowmap[r]]

    common = {
        "wz": np.ascontiguousarray(
            np.concatenate([Wd, Wc], axis=0).astype(bf)
        ),
        "wh": np.ascontiguousarray(whp.astype(bf)),
        "wo": np.ascontiguousarray(wop.astype(bf)),
        "bia": np.ascontiguousarray(bia),
        "boc": np.full((97, 1), np.asarray(b_out, np.float32).ravel()[0],
                       dtype=np.float32),
        "z": np.zeros((1, 640), dtype=ml_dtypes.bfloat16),
    }
    return common


def _pack_x(x_core, ctl_rows):
    b = x_core.shape[0]
    p = x_core.reshape(b, 32, 2, 32, 2).transpose(2, 4, 0, 1, 3)
    out = np.empty((7, b * N_NODES), dtype=ml_dtypes.bfloat16)
    out[0:4] = p.reshape(4, b * N_NODES).astype(ml_dtypes.bfloat16)
    out[4:7] = np.tile(ctl_rows, (1, b))
    return np.ascontiguousarray(out)


def _run(inputs, trace=False, tmpdir=None):
    x = np.asarray(inputs["x"], dtype=np.float32)
    common = _prep_weights(
        inputs["controls"], inputs["W_in"], inputs["b_in"], inputs["W_h"],
        inputs["b_h"], inputs["W_out"], inputs["b_out"],
        inputs["data_idx"], inputs["ctrl_idx"],
    )
    ctl_rows = (
        np.asarray(inputs["controls"], np.float32).T.astype(ml_dtypes.bfloat16)
    )

    nc = _build_program(B_CORE, CHUNK_B)

    core_ids = list(range(N_CORES))
    in_maps = []
    for i in core_ids:
        shard = _pack_x(x[i * B_CORE : (i + 1) * B_CORE, 0], ctl_rows)
        in_maps.append({"xs": shard, **common})

    res = run_bass_kernel_spmd(nc, in_maps, core_ids, trace=trace, tmpdir=tmpdir)
    out = np.concatenate([res.results[i]["out"] for i in core_ids], axis=0)
    return out.reshape(B, 32, 32), res.exec_time_ns


def kernel(**inputs):
    return _run(inputs, trace=False)[0]
